# revision 22
# baseline (speedup 1.0000x reference)
"""Causal single-head attention on 8 TRN2 NeuronCores — hybrid bf16/fp8.

Math (per batch b):
    qh = q @ (wq/8); kh = k @ wk; vh = v @ wv
    S^T[k,q] = kh qh^T            (scores transposed: k on partitions)
    P^T = exp(S^T - 2) * diagmask (constant bias keeps P in fp8e4 range;
                                   it cancels exactly in num/den)
    oT[d,q] = sum_k vh_ext[k,d]^T P^T[k,q]   with vh_ext = [vh | ones]
    row 64 of oT is the softmax denominator; host divides.

Sharding: 8 cores = 4 batches x 2 k-parities (flash-decoding style).
Core (b, p) handles batch b and the interleaved k-blocks {p, p+2, ...}
(128-row blocks): q-tile g sees 2g+2 local k-blocks; the last pair
crosses the diagonal and is masked.  Each core returns oT [65, 4096];
the host sums the parity partials and divides by the denominator row.

Perf structure (v5):
  * Inputs arrive as 24 large consumer-granule DMAs (contiguous 4-8KB
    per-partition lines) in exact consumption order on one HWDGE queue
    (the fp32 baseline issued 160 small DMAs at ~600ns of sequencer
    time each and was DMA-issue-bound).  Everything is SBUF-resident.
  * Mixed precision, validated against the max-abs-rel metric: the
    first q-tile / k-stage / v-stage (global rows < 512..1023, where
    softmax averaging is weakest) runs fully in bf16; later granules
    ship as fp8e4 and their projections contract chunk-PAIRS with
    DoubleRow matmuls (2 rows/cycle).  Scores stay bf16.  Attention*V
    uses one fp8 DoubleRow matmul per off-diagonal k-block pair
    (P and vh quantized to fp8 — softmax averaging damps this to
    ~1e-2 worst-case vs the 2e-2 gate); diagonal pairs stay bf16.
  * V-projection computes vh^T with the weight stationary (32 wide
    matmuls instead of 128 N=64 ones) and PE-transposes the result.
  * Within an attend the DIAGONAL pair is emitted first so its longer
    exp->mask->AV chain overlaps the other pairs' score matmuls
    instead of gating the next attend's PSUM reuse.
  * exp() on Activation; mask is a post-exp 0/1 multiply (Vector);
    output stores ride the GpSimd queue.  GpSimd cannot touch PSUM on
    TRN2, so all PSUM evictions stay on Vector.
"""

import sys

sys.path.insert(0, "/opt/trn_rl_repo")

import numpy as np
import ml_dtypes
from contextlib import ExitStack

import concourse.bass as bass
import concourse.mybir as mybir
import concourse.tile as tile
from concourse.bass_utils import run_bass_kernel_spmd

F32 = mybir.dt.float32
BF16 = mybir.dt.bfloat16
FP8 = mybir.dt.float8e4
FP8E3 = mybir.dt.float8e3
AF = mybir.ActivationFunctionType
DR = mybir.MatmulPerfMode.DoubleRow
BF16NP = ml_dtypes.bfloat16
FP8NP = ml_dtypes.float8_e4m3
FP8E3NP = ml_dtypes.float8_e3m4

B, S, E, D = 4, 4096, 1024, 64
NQT = S // 512          # 8 q-tiles of 512 rows
NST = 4                 # k/v staged in 4 chunks of 512 local rows
NKB_LOCAL = 16          # local (per-parity) 128-row k-blocks
EC = E // 128           # 8 e-chunks
DV = D + 1              # vh width incl. ones column
DVP = 128               # padded vh block pitch: [vh(64) | ones | 63 zeros]
                        # (dual-fp8 ldweights requires subtile M in {32,64,128})
EXP_BIAS = -2.0         # P' = exp(s-2): keeps P < 240 (fp8e4 max); cancels


def _patch_tile_drain():
    """Walrus in this container rejects >1 sync-wait on a Drain instruction.
    Spread the tail drain's waits across multiple drains (idempotent; the
    following all_engine_barrier orders everything)."""
    if getattr(tile.TileContext, "_drain_patched", False):
        return
    from concourse.tile import ScopedClock

    def _split_drain_and_barrier(self, tick_clock, wait_clock):
        drain_inst = self.nc.sync.drain()
        wait_clock.add_sem_waits(
            drain_inst.ins, ScopedClock({None: tick_clock.global_clock})
        )
        mi = drain_inst.ins
        si = mi.sync_info
        if si is not None and si.on_wait and len(si.on_wait) > 1:
            waits = list(si.on_wait)
            si.on_wait = waits[:1]
            for w in waits[1:]:
                d2 = self.nc.sync.drain().ins
                si2 = d2.sync_info
                if si2 is None:
                    d2.sync_info = mybir.SyncInfo(on_wait=[w], on_update=[])
                else:
                    si2.on_wait = list(si2.on_wait) + [w]
        self.nc.all_engine_barrier()
        assert self.sems is not None
        popped = self.nc._tile_sem_poison_stack.pop()
        assert popped is self._sem_poison
        self.nc.clear_and_free_semaphores(list(self.sems.allocated().values()))
        self.nc.all_engine_barrier()

    tile.TileContext._drain_and_barrier = _split_drain_and_barrier
    tile.TileContext._drain_patched = True


WAIT_LIMIT = 1


def _split_sync_waits(nc, limit=WAIT_LIMIT):
    """This container's walrus rejects instructions carrying more than ~limit
    sem waits. Hoist excess waits onto same-engine NoOps inserted just before
    the instruction (engine streams are in-order, so the waits still gate)."""
    n_nops = 0
    for f in nc.m.functions:
        for bb in f.blocks:
            il = bb.instructions
            i = 0
            while i < len(il):
                ins = il[i]
                si = ins.sync_info
                if si is not None and si.on_wait and len(si.on_wait) > limit:
                    waits = list(si.on_wait)
                    keep = waits[-limit:]
                    excess = waits[:-limit]
                    pos = i
                    for j in range(0, len(excess), limit):
                        nop = mybir.InstNoOp(
                            name=f"{ins.name}_wsplit{j}", ins=[], outs=[]
                        )
                        nop.engine = ins.engine
                        nop.sync_info = mybir.SyncInfo(
                            on_wait=excess[j : j + limit], on_update=[]
                        )
                        il.insert(pos, nop)
                        pos += 1
                        i += 1
                        n_nops += 1
                    si.on_wait = keep
                i += 1
    return n_nops


def _qdt(g):
    # q granules: tile 0 bf16; the rest fp8-e3m4 (4 mantissa bits — q input
    # error is a rank-64 perturbation of the row's whole score vector and
    # does NOT average away with depth, so q needs the extra mantissa)
    return BF16 if g == 0 else FP8E3


def _kdt(t):
    # k error DOES average away under softmax: e4m3 + DoubleRow is fine
    return BF16 if t == 0 else FP8


def _vdt(t):
    # v feeds the output directly; keep 2 stages in bf16
    return BF16 if t <= 1 else FP8


def _avf8(g):
    # fp8 DoubleRow AV only from q-tile 2 on (rows >= 1024)
    return g >= 2


def build_nc(extents, causal=True):
    """One SPMD program; per-core data differences live in the inputs.

    extents[g] = number of local 128-row k-blocks q-tile g attends to
    (always even: causal -> 2g+2, full -> 16).

    DRAM granules ([p, c*512+s] packing, c = e-chunk, s = row-in-granule):
      q{g} [128, 4096]  q-tile g    (g==0 bf16, else fp8)
      k{t} [128, 4096]  k-stage t   (t==0 bf16, else fp8)
      v{t} [128, 4096]  v-stage t   (t==0 bf16, else fp8)
    """
    _patch_tile_drain()
    nc = bass.Bass("TRN2", target_bir_lowering=False)

    qd = [nc.dram_tensor(f"q{g}", [128, 4096], _qdt(g), kind="ExternalInput")
          for g in range(NQT)]
    kd = [nc.dram_tensor(f"k{t}", [128, 4096], _kdt(t), kind="ExternalInput")
          for t in range(NST)]
    vd = [nc.dram_tensor(f"v{t}", [128, 4096], _vdt(t), kind="ExternalInput")
          for t in range(NST)]
    # bf16 weights, chunked: w_r[p, c*D+d] = w[c*128+p, d] (q/k duplicated)
    wq = nc.dram_tensor("wq", [128, EC * 2 * D], BF16, kind="ExternalInput")
    wk = nc.dram_tensor("wk", [128, EC * 2 * D], BF16, kind="ExternalInput")
    wv = nc.dram_tensor("wv", [128, EC * D], BF16, kind="ExternalInput")
    # fp8 copies (same layout; DoubleRow consumes chunk pairs)
    wq8 = nc.dram_tensor("wq8", [128, EC * 2 * D], FP8, kind="ExternalInput")
    wk8 = nc.dram_tensor("wk8", [128, EC * 2 * D], FP8, kind="ExternalInput")
    wv8 = nc.dram_tensor("wv8", [128, EC * D], FP8, kind="ExternalInput")
    idn = nc.dram_tensor("idn", [64, 64], BF16, kind="ExternalInput")
    msk = nc.dram_tensor("msk", [128, 1024], BF16, kind="ExternalInput")
    o = nc.dram_tensor("o", [DV, S], F32, kind="ExternalOutput")

    with tile.TileContext(nc) as tc, ExitStack() as ctx:
        const = ctx.enter_context(tc.tile_pool(name="const", bufs=1))
        xin = ctx.enter_context(tc.tile_pool(name="xin", bufs=1))
        big = ctx.enter_context(tc.tile_pool(name="big", bufs=1))
        ptp = ctx.enter_context(tc.tile_pool(name="ptp", bufs=2))
        ptp8 = ctx.enter_context(tc.tile_pool(name="ptp8", bufs=3))
        obp = ctx.enter_context(tc.tile_pool(name="obp", bufs=2))

        psP = ctx.enter_context(tc.tile_pool(name="psP", bufs=1, space="PSUM"))
        psS = ctx.enter_context(tc.tile_pool(name="psS", bufs=2, space="PSUM"))
        psO = ctx.enter_context(tc.tile_pool(name="psO", bufs=1, space="PSUM"))
        psT = ctx.enter_context(tc.tile_pool(name="psT", bufs=1, space="PSUM"))
        psX = ctx.enter_context(tc.tile_pool(name="psX", bufs=1, space="PSUM"))

        # --- SBUF homes -------------------------------------------------
        w_sb = {}
        for name, dram, wd, dt_ in (
            ("wk", wk, 2, BF16), ("wq", wq, 2, BF16), ("wv", wv, 1, BF16),
            ("wk8", wk8, 2, FP8), ("wq8", wq8, 2, FP8), ("wv8", wv8, 1, FP8),
        ):
            w_sb[name] = const.tile(
                [128, EC * wd * D], dt_, tag=f"w_{name}", name=f"w_{name}_sb"
            )
        idn_sb = const.tile([64, 64], BF16, tag="idn")
        msk_sb = const.tile([128, 1024], BF16, tag="msk")
        bias_sb = const.tile([128, 1], F32, tag="bias")
        nc.vector.memset(bias_sb[:], EXP_BIAS)
        q_sb = [
            xin.tile([128, 4096], _qdt(g), tag=f"q{g}", name=f"q{g}_sb")
            for g in range(NQT)
        ]
        k_sb = [
            xin.tile([128, 4096], _kdt(t), tag=f"k{t}", name=f"k{t}_sb")
            for t in range(NST)
        ]
        v_sb = [
            xin.tile([128, 4096], _vdt(t), tag=f"v{t}", name=f"v{t}_sb")
            for t in range(NST)
        ]

        # --- input DMAs, one per granule, in consumption order ----------
        dma = nc.sync.dma_start
        dma(w_sb["wk"][:], wk[:])
        dma(w_sb["wq"][:], wq[:])
        if causal:
            dma(k_sb[0][:], kd[0][:])
            dma(q_sb[0][:], qd[0][:])
            dma(msk_sb[:], msk[:])
            dma(w_sb["wv"][:], wv[:])
            dma(idn_sb[:], idn[:])
            dma(v_sb[0][:], vd[0][:])
            dma(w_sb["wq8"][:], wq8[:])
            dma(q_sb[1][:], qd[1][:])
            dma(w_sb["wk8"][:], wk8[:])
            dma(w_sb["wv8"][:], wv8[:])
            for st in range(1, NST):
                dma(k_sb[st][:], kd[st][:])
                dma(v_sb[st][:], vd[st][:])
                dma(q_sb[2 * st][:], qd[2 * st][:])
                dma(q_sb[2 * st + 1][:], qd[2 * st + 1][:])
        else:
            dma(w_sb["wv"][:], wv[:])
            dma(idn_sb[:], idn[:])
            dma(msk_sb[:], msk[:])
            dma(w_sb["wk8"][:], wk8[:])
            dma(w_sb["wq8"][:], wq8[:])
            dma(w_sb["wv8"][:], wv8[:])
            for t in range(NST):
                dma(k_sb[t][:], kd[t][:])
                dma(v_sb[t][:], vd[t][:])
            for g in range(NQT):
                dma(q_sb[g][:], qd[g][:])

        # [p, c, s] views of the packed granules
        qv = [t[:].rearrange("p (c s) -> p c s", s=512) for t in q_sb]
        kv = [t[:].rearrange("p (c s) -> p c s", s=512) for t in k_sb]
        vv = [t[:].rearrange("p (c s) -> p c s", s=512) for t in v_sb]

        qhT_sb = big.tile([128, S], BF16, tag="qhT")
        khT_sb = big.tile([128, S // 2], BF16, tag="khT")
        vhT_sb = big.tile([64, NKB_LOCAL * 128], BF16, tag="vhT")
        vh_sb = big.tile([128, NKB_LOCAL * DVP], BF16, tag="vh")
        vh8_sb = big.tile([128, NKB_LOCAL * DVP], FP8, tag="vh8")
        # col D = ones (softmax denominator via the AV matmul); cols D+1..
        # are never read but are zeroed so stale fp8 bytes can't be NaN/inf
        for t in (vh_sb, vh8_sb):
            v3 = t[:].rearrange("p (b c) -> p b c", c=DVP)
            nc.gpsimd.memset(v3[:, :, D], 1.0)
            nc.gpsimd.memset(v3[:, :, D + 1 :], 0.0)

        # ---- projections as closure lists so they can be interleaved into
        # ---- attend pair slots (keeps the PE dense while ACT runs exp) ----
        fill_q = []

        def drain(n=None):
            k2 = 0
            while fill_q and (n is None or k2 < n):
                fill_q.pop(0)()
                k2 += 1

        def inline(cls):
            for f in cls:
                f()

        def qk_closures(i, src_v, wtag, outT_sb, col0):
            # outT[128, col0:+512] = (x @ [w|w])^T, contracting E.
            # q fp8 granules are e3m4: plain matmuls against e4m3 weights
            # (e3m4 has no DoubleRow); k fp8 granules are e4m3: chunk-pair
            # DoubleRow.
            dt_ = _qdt(i) if wtag == "wq" else _kdt(i)
            cell = {}
            cls = []
            if dt_ != FP8:
                w = w_sb[wtag] if dt_ == BF16 else w_sb[wtag + "8"]

                def mk(c):
                    def f():
                        if c == 0:
                            cell["ps"] = psP.tile([128, 512], F32, tag="psP", name="psp_t")
                        nc.tensor.matmul(
                            cell["ps"][:],
                            lhsT=w[:, c * 2 * D : (c + 1) * 2 * D],
                            rhs=src_v[:, c, :],
                            start=(c == 0),
                            stop=(c == EC - 1),
                        )
                        if c == EC - 1:
                            nc.vector.tensor_copy(
                                outT_sb[:, col0 : col0 + 512], cell["ps"][:]
                            )
                    return f

                cls = [mk(c) for c in range(EC)]
            else:
                w = w_sb[wtag + "8"]

                def mk8(ci):
                    def f():
                        if ci == 0:
                            cell["ps"] = psP.tile([128, 512], F32, tag="psP", name="psp_t")
                        nc.tensor.matmul(
                            cell["ps"][:],
                            lhsT=w[:, ci * 4 * D : (ci + 1) * 4 * D].rearrange(
                                "p (two f) -> p two f", two=2
                            ),
                            rhs=src_v[:, 2 * ci : 2 * ci + 2, :],
                            start=(ci == 0),
                            stop=(ci == EC // 2 - 1),
                            perf_mode=DR,
                        )
                        if ci == EC // 2 - 1:
                            nc.vector.tensor_copy(
                                outT_sb[:, col0 : col0 + 512], cell["ps"][:]
                            )
                    return f

                cls = [mk8(ci) for ci in range(EC // 2)]
            return cls

        def v_closures(st):
            # vh^T[d, krow] for stage st with wv stationary, then PE-transpose
            # each 128-row block into vh[krow, d] (bf16 + fp8 copies)
            cell = {}
            cls = []
            if _vdt(st) == BF16:

                def mkv(c):
                    def f():
                        if c == 0:
                            cell["ps"] = psT.tile([64, 512], F32, tag="psT", name="pst_t")
                        nc.tensor.matmul(
                            cell["ps"][:],
                            lhsT=w_sb["wv"][:, c * D : (c + 1) * D],
                            rhs=vv[st][:, c, :],
                            start=(c == 0),
                            stop=(c == EC - 1),
                        )
                        if c == EC - 1:
                            nc.vector.tensor_copy(
                                vhT_sb[:, st * 512 : st * 512 + 512], cell["ps"][:]
                            )
                    return f

                cls = [mkv(c) for c in range(EC)]
            else:

                def mkv8(ci):
                    def f():
                        if ci == 0:
                            cell["ps"] = psT.tile([64, 512], F32, tag="psT", name="pst_t")
                        nc.tensor.matmul(
                            cell["ps"][:],
                            lhsT=w_sb["wv8"][
                                :, ci * 2 * D : (ci + 1) * 2 * D
                            ].rearrange("p (two f) -> p two f", two=2),
                            rhs=vv[st][:, 2 * ci : 2 * ci + 2, :],
                            start=(ci == 0),
                            stop=(ci == EC // 2 - 1),
                            perf_mode=DR,
                        )
                        if ci == EC // 2 - 1:
                            nc.vector.tensor_copy(
                                vhT_sb[:, st * 512 : st * 512 + 512], cell["ps"][:]
                            )
                    return f

                cls = [mkv8(ci) for ci in range(EC // 2)]

            def mkt(jj):
                def f():
                    blk = 4 * st + jj
                    px = psX.tile([128, 64], BF16, tag="psX")
                    nc.tensor.matmul(
                        px[:],
                        lhsT=vhT_sb[:, blk * 128 : (blk + 1) * 128],
                        rhs=idn_sb[:],
                        is_transpose=True,
                    )
                    nc.vector.tensor_copy(
                        vh_sb[:, blk * DVP : blk * DVP + D], px[:]
                    )
                    nc.vector.tensor_copy(
                        vh8_sb[:, blk * DVP : blk * DVP + D], px[:]
                    )
                return f

            cls += [mkt(jj) for jj in range(4)]
            return cls

        def warmup(n):
            # dummy matmuls on a zeroed scratch tile: ramps the PE p-state
            # (full clock needs ~3us of continuous busy) while the first
            # input granules stream in
            sc = ptp.tile([128, 1024], BF16, tag="pt")
            nc.vector.memset(sc[:], 0.0)
            ps = psS.tile([128, 1024], F32, tag="psS")
            for _ in range(n):
                nc.tensor.matmul(
                    ps[:, 0:512],
                    lhsT=sc[:, 0:128],
                    rhs=sc[:, 0:512],
                    start=True,
                    stop=True,
                )

        def attend(g, mid=None, fill=0):
            npairs = extents[g] // 2
            ps_o = psO.tile([128, 512], F32, tag="psO")
            qlo = qhT_sb[0:64, g * 512 : (g + 1) * 512]
            qhi = qhT_sb[64:128, g * 512 : (g + 1) * 512]
            # diagonal pair first: its exp->mask->AV latency overlaps the
            # other pairs' score matmuls instead of gating the next attend
            order = [npairs - 1] + list(range(npairs - 1))
            for idx, pr in enumerate(order):
                first = idx == 0
                last = idx == len(order) - 1
                diag = causal and pr == npairs - 1
                # q-tile 0 projects through bf16 wq (pre-scaled 1/sqrt(D));
                # fp8 tiles use wq8 = 8*wq (keeps e4m3 out of subnormals),
                # so fold 1/(8*sqrt(D)) into the exp scale
                esc = 1.0 if _qdt(g) == BF16 else 1.0 / (8.0 * np.sqrt(D))
                use_bf = diag or not causal or not _avf8(g)
                ps_s = psS.tile([128, 1024], F32, tag="psS")
                for h in range(2):
                    l = 2 * pr + h
                    krows = khT_sb[0:64, :] if h == 0 else khT_sb[64:128, :]
                    nc.tensor.matmul(
                        ps_s[:, h * 512 : (h + 1) * 512],
                        lhsT=krows[:, l * 128 : (l + 1) * 128],
                        rhs=(qlo if h == 0 else qhi),
                        start=True,
                        stop=True,
                    )
                if use_bf:
                    # bf16 path (mask applies post-exp as 0/1 multiply)
                    pt = ptp.tile([128, 1024], BF16, tag="pt")
                    nc.scalar.activation(
                        pt[:], ps_s[:], AF.Exp, bias=bias_sb[:], scale=esc
                    )
                    if diag:
                        nc.vector.tensor_mul(pt[:], pt[:], msk_sb[:])
                    if mid is not None and idx == 0:
                        mid()
                        mid = None
                    drain(fill)
                    for h in range(2):
                        l = 2 * pr + h
                        nc.tensor.matmul(
                            ps_o[:],
                            lhsT=vh_sb[:, l * DVP : (l + 1) * DVP],
                            rhs=pt[:, h * 512 : (h + 1) * 512],
                            start=(first and h == 0),
                            stop=(last and h == 1),
                        )
                else:
                    # off-diagonal: fp8 P, one DoubleRow AV for the pair
                    pt8 = ptp8.tile([128, 1024], FP8, tag="pt8")
                    nc.scalar.activation(
                        pt8[:], ps_s[:], AF.Exp, bias=bias_sb[:], scale=esc
                    )
                    drain(fill)
                    nc.tensor.matmul(
                        ps_o[:],
                        lhsT=vh8_sb[
                            :, (2 * pr) * DVP : (2 * pr + 2) * DVP
                        ].rearrange("p (two f) -> p two f", two=2),
                        rhs=pt8[:].rearrange("p (two f) -> p two f", two=2),
                        start=first,
                        stop=last,
                        perf_mode=DR,
                    )
            ob = obp.tile([65, 512], F32, tag="ob")
            nc.vector.tensor_copy(ob[:], ps_o[0:65, :])
            nc.gpsimd.dma_start(o[:, g * 512 : (g + 1) * 512], ob[:])

        def push_stage(st):
            # stage st's kh/vh projections + its two q-tile projections
            fill_q.extend(qk_closures(st, kv[st], "wk", khT_sb, st * 512))
            fill_q.extend(v_closures(st))
            for g in (2 * st, 2 * st + 1):
                fill_q.extend(qk_closures(g, qv[g], "wq", qhT_sb, g * 512))

        if causal:
            # Warm the PE while wk/wq/k0/q0 stream in, then get the first
            # exp onto Activation ASAP: kh0/qh0/scores(0) before the
            # V-projection (whose v0 DMA lands later); V-proj slots between
            # exp(0) and AV(0).  From there, each stage's projections are
            # drained as PE fillers inside the previous attends' exp slots
            # so the Activation stream never waits on a projection burst.
            warmup(12)
            inline(qk_closures(0, kv[0], "wk", khT_sb, 0))
            inline(qk_closures(0, qv[0], "wq", qhT_sb, 0))
            attend(0, mid=lambda: inline(v_closures(0)))
            inline(qk_closures(1, qv[1], "wq", qhT_sb, 512))
            push_stage(1)
            attend(1, fill=3)
            drain()
            push_stage(2)
            attend(2, fill=3)
            attend(3, fill=3)
            drain()
            push_stage(3)
            attend(4, fill=3)
            attend(5, fill=3)
            drain()
            attend(6)
            attend(7)
        else:
            for st in range(NST):
                inline(qk_closures(st, kv[st], "wk", khT_sb, st * 512))
                inline(v_closures(st))
            for g in range(NQT):
                inline(qk_closures(g, qv[g], "wq", qhT_sb, g * 512))
                attend(g)

    _split_sync_waits(nc)
    return nc


_CACHE = {}


def _get_nc(causal):
    key = bool(causal)
    if key not in _CACHE:
        extents = [2 * g + 2 for g in range(NQT)] if causal else [NKB_LOCAL] * NQT
        _CACHE[key] = build_nc(extents, causal=key)
    return _CACHE[key]


def _pack(block512, np_dt):
    # [512, E] -> [128, 4096] with out[p, c*512+s] = block[s, c*128+p]
    return np.ascontiguousarray(
        block512.reshape(512, EC, 128).transpose(2, 1, 0).reshape(128, 4096)
    ).astype(np_dt)


def _np_qdt(g):
    return BF16NP if g == 0 else FP8E3NP


def _np_kdt(t):
    return BF16NP if t == 0 else FP8NP


def _np_vdt(t):
    return BF16NP if t <= 1 else FP8NP


def kernel(q, k, v, mask, wq, wk, wv):
    q = np.asarray(q, np.float32)
    k = np.asarray(k, np.float32)
    v = np.asarray(v, np.float32)
    mask = np.asarray(mask)
    wq = np.asarray(wq, np.float32)
    wk = np.asarray(wk, np.float32)
    wv = np.asarray(wv, np.float32)

    m0 = mask[0]
    causal = bool(m0[0, 1] == 0)
    tril = np.tril(np.ones((S, S), np.int32))
    if causal:
        ok = np.array_equal(m0.astype(np.int32), tril)
    else:
        ok = bool((m0 != 0).all())
    if not ok:
        # arbitrary mask: bail out to exact numpy (correctness safety net)
        qh = q @ wq
        kh = k @ wk
        vh = v @ wv
        s = np.einsum("bqd,bkd->bqk", qh, kh) / np.sqrt(D)
        s = np.where(mask == 0, -np.inf, s)
        s = s - s.max(-1, keepdims=True)
        p = np.exp(s)
        p /= p.sum(-1, keepdims=True)
        return np.einsum("bqk,bkd->bqd", p, vh).astype(np.float32)

    nc = _get_nc(causal)

    def wchunk(w, dup, np_dt):
        # [E, D] -> [128, EC*(2)D] with w_r[p, c*D+d] = w[c*128+p, d]
        r = w.reshape(EC, 128, D).transpose(1, 0, 2)
        if dup:
            r = np.concatenate([r, r], axis=2)
        return np.ascontiguousarray(r.reshape(128, -1)).astype(np_dt)

    consts = {
        "wq": wchunk(wq / np.sqrt(D), True, BF16NP),
        "wk": wchunk(wk, True, BF16NP),
        "wv": wchunk(wv, False, BF16NP),
        # wq8 is scaled UP by 8 (not down by 1/sqrt(D)): wq/8 — and even
        # raw wq — sits partly in e4m3's subnormal range.  The 1/(8*8) is
        # folded into the exp() scale for fp8-projected q-tiles.
        "wq8": wchunk(wq * 8.0, True, FP8NP),
        "wk8": wchunk(wk, True, FP8NP),
        "wv8": wchunk(wv, False, FP8NP),
        "idn": np.eye(64, dtype=BF16NP),
    }

    in_maps = []
    for b in range(B):
        qt = {
            f"q{g}": _pack(q[b][g * 512 : (g + 1) * 512], _np_qdt(g))
            for g in range(NQT)
        }
        for p in range(2):
            kb = k[b].reshape(32, 128, E)[p::2]
            vb = v[b].reshape(32, 128, E)[p::2]
            kt = {
                f"k{t}": _pack(kb[4 * t : 4 * t + 4].reshape(512, E), _np_kdt(t))
                for t in range(NST)
            }
            vt = {
                f"v{t}": _pack(vb[4 * t : 4 * t + 4].reshape(512, E), _np_vdt(t))
                for t in range(NST)
            }
            if causal:
                kk = np.arange(128)[:, None]
                qq = np.arange(512)[None, :]
                parts = []
                for j in (p, p + 2):
                    allowed = qq >= (j * 128 + kk)
                    parts.append(np.where(allowed, 1.0, 0.0).astype(BF16NP))
                mskd = np.concatenate(parts, axis=1)  # [128, 1024] of 1/0
            else:
                mskd = np.ones((128, 1024), BF16NP)
            im = dict(consts)
            im["msk"] = mskd
            im.update(qt)
            im.update(kt)
            im.update(vt)
            in_maps.append(im)

    globals()["_last_in_maps"] = in_maps
    res = run_bass_kernel_spmd(nc, in_maps, core_ids=list(range(8)))

    out = np.empty((B, S, D), np.float32)
    for b in range(B):
        oe = res.results[2 * b]["o"]    # [65, 4096]
        oo = res.results[2 * b + 1]["o"]
        num = oe[:D] + oo[:D]           # [64, 4096]
        den = oe[D] + oo[D]             # [4096]
        out[b] = (num / den).T
    return out


# revision 23
# speedup vs baseline: 1.1743x; 1.1743x over previous
"""Causal single-head attention on 8 TRN2 NeuronCores — hybrid bf16/fp8.

Math (per batch b):
    qh = q @ (wq/8); kh = k @ wk; vh = v @ wv
    S^T[k,q] = kh qh^T            (scores transposed: k on partitions)
    P^T = exp(S^T - 2) * diagmask (constant bias keeps P in fp8e4 range;
                                   it cancels exactly in num/den)
    oT[d,q] = sum_k vh_ext[k,d]^T P^T[k,q]   with vh_ext = [vh | ones]
    row 64 of oT is the softmax denominator; host divides.

Sharding: 8 cores = 4 batches x 2 k-parities (flash-decoding style).
Core (b, p) handles batch b and the interleaved k-blocks {p, p+2, ...}
(128-row blocks): q-tile g sees 2g+2 local k-blocks; the last pair
crosses the diagonal and is masked.  Each core returns oT [65, 4096];
the host sums the parity partials and divides by the denominator row.

Perf structure (v5):
  * Inputs arrive as 24 large consumer-granule DMAs (contiguous 4-8KB
    per-partition lines) in exact consumption order on one HWDGE queue
    (the fp32 baseline issued 160 small DMAs at ~600ns of sequencer
    time each and was DMA-issue-bound).  Everything is SBUF-resident.
  * Mixed precision, validated against the max-abs-rel metric: the
    first q-tile / k-stage / v-stage (global rows < 512..1023, where
    softmax averaging is weakest) runs fully in bf16; later granules
    ship as fp8e4 and their projections contract chunk-PAIRS with
    DoubleRow matmuls (2 rows/cycle).  Scores stay bf16.  Attention*V
    uses one fp8 DoubleRow matmul per off-diagonal k-block pair
    (P and vh quantized to fp8 — softmax averaging damps this to
    ~1e-2 worst-case vs the 2e-2 gate); diagonal pairs stay bf16.
  * V-projection computes vh^T with the weight stationary (32 wide
    matmuls instead of 128 N=64 ones) and PE-transposes the result.
  * Within an attend the DIAGONAL pair is emitted first so its longer
    exp->mask->AV chain overlaps the other pairs' score matmuls
    instead of gating the next attend's PSUM reuse.
  * exp() on Activation; mask is a post-exp 0/1 multiply (Vector);
    output stores ride the GpSimd queue.  GpSimd cannot touch PSUM on
    TRN2, so all PSUM evictions stay on Vector.
"""

import sys

sys.path.insert(0, "/opt/trn_rl_repo")

import numpy as np
import ml_dtypes
from contextlib import ExitStack

import concourse.bass as bass
import concourse.mybir as mybir
import concourse.tile as tile
from concourse.bass_utils import run_bass_kernel_spmd

F32 = mybir.dt.float32
BF16 = mybir.dt.bfloat16
FP8 = mybir.dt.float8e4
FP8E3 = mybir.dt.float8e3
AF = mybir.ActivationFunctionType
DR = mybir.MatmulPerfMode.DoubleRow
BF16NP = ml_dtypes.bfloat16
FP8NP = ml_dtypes.float8_e4m3
FP8E3NP = ml_dtypes.float8_e3m4

B, S, E, D = 4, 4096, 1024, 64
NQT = S // 512          # 8 q-tiles of 512 rows
NST = 4                 # k/v staged in 4 chunks of 512 local rows
NKB_LOCAL = 16          # local (per-parity) 128-row k-blocks
EC = E // 128           # 8 e-chunks
DV = D + 1              # vh width incl. ones column
DVP = 128               # padded vh block pitch: [vh(64) | ones | 63 zeros]
                        # (dual-fp8 ldweights requires subtile M in {32,64,128})
EXP_BIAS = -2.0         # P' = exp(s-2): keeps P < 240 (fp8e4 max); cancels


def _patch_tile_drain():
    """Walrus in this container rejects >1 sync-wait on a Drain instruction.
    Spread the tail drain's waits across multiple drains (idempotent; the
    following all_engine_barrier orders everything)."""
    if getattr(tile.TileContext, "_drain_patched", False):
        return
    from concourse.tile import ScopedClock

    def _split_drain_and_barrier(self, tick_clock, wait_clock):
        drain_inst = self.nc.sync.drain()
        wait_clock.add_sem_waits(
            drain_inst.ins, ScopedClock({None: tick_clock.global_clock})
        )
        mi = drain_inst.ins
        si = mi.sync_info
        if si is not None and si.on_wait and len(si.on_wait) > 1:
            waits = list(si.on_wait)
            si.on_wait = waits[:1]
            for w in waits[1:]:
                d2 = self.nc.sync.drain().ins
                si2 = d2.sync_info
                if si2 is None:
                    d2.sync_info = mybir.SyncInfo(on_wait=[w], on_update=[])
                else:
                    si2.on_wait = list(si2.on_wait) + [w]
        self.nc.all_engine_barrier()
        assert self.sems is not None
        popped = self.nc._tile_sem_poison_stack.pop()
        assert popped is self._sem_poison
        self.nc.clear_and_free_semaphores(list(self.sems.allocated().values()))
        self.nc.all_engine_barrier()

    tile.TileContext._drain_and_barrier = _split_drain_and_barrier
    tile.TileContext._drain_patched = True


WAIT_LIMIT = 1


def _split_sync_waits(nc, limit=WAIT_LIMIT):
    """This container's walrus rejects instructions carrying more than ~limit
    sem waits. Hoist excess waits onto same-engine NoOps inserted just before
    the instruction (engine streams are in-order, so the waits still gate)."""
    n_nops = 0
    for f in nc.m.functions:
        for bb in f.blocks:
            il = bb.instructions
            i = 0
            while i < len(il):
                ins = il[i]
                si = ins.sync_info
                if si is not None and si.on_wait and len(si.on_wait) > limit:
                    waits = list(si.on_wait)
                    keep = waits[-limit:]
                    excess = waits[:-limit]
                    pos = i
                    for j in range(0, len(excess), limit):
                        nop = mybir.InstNoOp(
                            name=f"{ins.name}_wsplit{j}", ins=[], outs=[]
                        )
                        nop.engine = ins.engine
                        nop.sync_info = mybir.SyncInfo(
                            on_wait=excess[j : j + limit], on_update=[]
                        )
                        il.insert(pos, nop)
                        pos += 1
                        i += 1
                        n_nops += 1
                    si.on_wait = keep
                i += 1
    return n_nops


def _qdt(g):
    # q granules: tile 0 bf16; the rest fp8-e3m4 (4 mantissa bits — q input
    # error is a rank-64 perturbation of the row's whole score vector and
    # does NOT average away with depth, so q needs the extra mantissa)
    return BF16 if g == 0 else FP8E3


def _kdt(t):
    # k error DOES average away under softmax: e4m3 + DoubleRow is fine
    return BF16 if t == 0 else FP8


def _vdt(t):
    # v feeds the output directly; keep 2 stages in bf16
    return BF16 if t <= 1 else FP8


def _avf8(g):
    # fp8 DoubleRow AV only from q-tile 2 on (rows >= 1024)
    return g >= 2


def build_nc(extents, causal=True):
    """One SPMD program; per-core data differences live in the inputs.

    extents[g] = number of local 128-row k-blocks q-tile g attends to
    (always even: causal -> 2g+2, full -> 16).

    DRAM granules ([p, c*512+s] packing, c = e-chunk, s = row-in-granule):
      q{g} [128, 4096]  q-tile g    (g==0 bf16, else fp8)
      k{t} [128, 4096]  k-stage t   (t==0 bf16, else fp8)
      v{t} [128, 4096]  v-stage t   (t==0 bf16, else fp8)
    """
    _patch_tile_drain()
    nc = bass.Bass("TRN2", target_bir_lowering=False)

    qd = [nc.dram_tensor(f"q{g}", [128, 4096], _qdt(g), kind="ExternalInput")
          for g in range(NQT)]
    kd = [nc.dram_tensor(f"k{t}", [128, 4096], _kdt(t), kind="ExternalInput")
          for t in range(NST)]
    vd = [nc.dram_tensor(f"v{t}", [128, 4096], _vdt(t), kind="ExternalInput")
          for t in range(NST)]
    # bf16 weights, chunked: w_r[p, c*D+d] = w[c*128+p, d] (q/k duplicated)
    wq = nc.dram_tensor("wq", [128, EC * 2 * D], BF16, kind="ExternalInput")
    wk = nc.dram_tensor("wk", [128, EC * 2 * D], BF16, kind="ExternalInput")
    wv = nc.dram_tensor("wv", [128, EC * D], BF16, kind="ExternalInput")
    # fp8 copies (same layout; DoubleRow consumes chunk pairs)
    wq8 = nc.dram_tensor("wq8", [128, EC * 2 * D], FP8, kind="ExternalInput")
    wk8 = nc.dram_tensor("wk8", [128, EC * 2 * D], FP8, kind="ExternalInput")
    wv8 = nc.dram_tensor("wv8", [128, EC * D], FP8, kind="ExternalInput")
    idn = nc.dram_tensor("idn", [64, 64], BF16, kind="ExternalInput")
    msk = nc.dram_tensor("msk", [128, 1024], BF16, kind="ExternalInput")
    o = nc.dram_tensor("o", [DV, S], F32, kind="ExternalOutput")

    with tile.TileContext(nc) as tc, ExitStack() as ctx:
        const = ctx.enter_context(tc.tile_pool(name="const", bufs=1))
        xin = ctx.enter_context(tc.tile_pool(name="xin", bufs=1))
        big = ctx.enter_context(tc.tile_pool(name="big", bufs=1))
        ptp = ctx.enter_context(tc.tile_pool(name="ptp", bufs=2))
        ptp8 = ctx.enter_context(tc.tile_pool(name="ptp8", bufs=3))
        obp = ctx.enter_context(tc.tile_pool(name="obp", bufs=2))

        psP = ctx.enter_context(tc.tile_pool(name="psP", bufs=1, space="PSUM"))
        psS = ctx.enter_context(tc.tile_pool(name="psS", bufs=2, space="PSUM"))
        psO = ctx.enter_context(tc.tile_pool(name="psO", bufs=1, space="PSUM"))
        psT = ctx.enter_context(tc.tile_pool(name="psT", bufs=1, space="PSUM"))
        psX = ctx.enter_context(tc.tile_pool(name="psX", bufs=1, space="PSUM"))

        # --- SBUF homes -------------------------------------------------
        w_sb = {}
        for name, dram, wd, dt_ in (
            ("wk", wk, 2, BF16), ("wq", wq, 2, BF16), ("wv", wv, 1, BF16),
            ("wk8", wk8, 2, FP8), ("wq8", wq8, 2, FP8), ("wv8", wv8, 1, FP8),
        ):
            w_sb[name] = const.tile(
                [128, EC * wd * D], dt_, tag=f"w_{name}", name=f"w_{name}_sb"
            )
        idn_sb = const.tile([64, 64], BF16, tag="idn")
        msk_sb = const.tile([128, 1024], BF16, tag="msk")
        bias_sb = const.tile([128, 1], F32, tag="bias")
        nc.vector.memset(bias_sb[:], EXP_BIAS)
        q_sb = [
            xin.tile([128, 4096], _qdt(g), tag=f"q{g}", name=f"q{g}_sb")
            for g in range(NQT)
        ]
        k_sb = [
            xin.tile([128, 4096], _kdt(t), tag=f"k{t}", name=f"k{t}_sb")
            for t in range(NST)
        ]
        v_sb = [
            xin.tile([128, 4096], _vdt(t), tag=f"v{t}", name=f"v{t}_sb")
            for t in range(NST)
        ]

        # --- input DMAs, one per granule, in consumption order ----------
        dma = nc.sync.dma_start
        dma(w_sb["wk"][:], wk[:])
        dma(w_sb["wq"][:], wq[:])
        if causal:
            dma(k_sb[0][:], kd[0][:])
            dma(q_sb[0][:], qd[0][:])
            dma(msk_sb[:], msk[:])
            dma(w_sb["wv"][:], wv[:])
            dma(idn_sb[:], idn[:])
            dma(v_sb[0][:], vd[0][:])
            dma(w_sb["wq8"][:], wq8[:])
            dma(q_sb[1][:], qd[1][:])
            dma(w_sb["wk8"][:], wk8[:])
            dma(w_sb["wv8"][:], wv8[:])
            for st in range(1, NST):
                dma(k_sb[st][:], kd[st][:])
                dma(v_sb[st][:], vd[st][:])
                dma(q_sb[2 * st][:], qd[2 * st][:])
                dma(q_sb[2 * st + 1][:], qd[2 * st + 1][:])
        else:
            dma(w_sb["wv"][:], wv[:])
            dma(idn_sb[:], idn[:])
            dma(msk_sb[:], msk[:])
            dma(w_sb["wk8"][:], wk8[:])
            dma(w_sb["wq8"][:], wq8[:])
            dma(w_sb["wv8"][:], wv8[:])
            for t in range(NST):
                dma(k_sb[t][:], kd[t][:])
                dma(v_sb[t][:], vd[t][:])
            for g in range(NQT):
                dma(q_sb[g][:], qd[g][:])

        # [p, c, s] views of the packed granules
        qv = [t[:].rearrange("p (c s) -> p c s", s=512) for t in q_sb]
        kv = [t[:].rearrange("p (c s) -> p c s", s=512) for t in k_sb]
        vv = [t[:].rearrange("p (c s) -> p c s", s=512) for t in v_sb]

        qhT_sb = big.tile([128, S], BF16, tag="qhT")
        khT_sb = big.tile([128, S // 2], BF16, tag="khT")
        vhT_sb = big.tile([64, NKB_LOCAL * 128], BF16, tag="vhT")
        vh_sb = big.tile([128, NKB_LOCAL * DVP], BF16, tag="vh")
        vh8_sb = big.tile([128, NKB_LOCAL * DVP], FP8, tag="vh8")
        # col D = ones (softmax denominator via the AV matmul); cols D+1..
        # are never read but are zeroed so stale fp8 bytes can't be NaN/inf
        for t in (vh_sb, vh8_sb):
            v3 = t[:].rearrange("p (b c) -> p b c", c=DVP)
            nc.gpsimd.memset(v3[:, :, D], 1.0)
            nc.gpsimd.memset(v3[:, :, D + 1 :], 0.0)

        # ---- projections as closure lists so they can be interleaved into
        # ---- attend pair slots (keeps the PE dense while ACT runs exp) ----
        fill_q = []

        def drain(n=None):
            k2 = 0
            while fill_q and (n is None or k2 < n):
                fill_q.pop(0)()
                k2 += 1

        def inline(cls):
            for f in cls:
                f()

        def qk_closures(i, src_v, wtag, outT_sb, col0):
            # outT[128, col0:+512] = (x @ [w|w])^T, contracting E.
            # q fp8 granules are e3m4: plain matmuls against e4m3 weights
            # (e3m4 has no DoubleRow); k fp8 granules are e4m3: chunk-pair
            # DoubleRow.
            dt_ = _qdt(i) if wtag == "wq" else _kdt(i)
            cell = {}
            cls = []
            if dt_ != FP8:
                w = w_sb[wtag] if dt_ == BF16 else w_sb[wtag + "8"]

                def mk(c):
                    def f():
                        if c == 0:
                            cell["ps"] = psP.tile([128, 512], F32, tag="psP", name="psp_t")
                        nc.tensor.matmul(
                            cell["ps"][:],
                            lhsT=w[:, c * 2 * D : (c + 1) * 2 * D],
                            rhs=src_v[:, c, :],
                            start=(c == 0),
                            stop=(c == EC - 1),
                        )
                        if c == EC - 1:
                            nc.vector.tensor_copy(
                                outT_sb[:, col0 : col0 + 512], cell["ps"][:]
                            )
                    return f

                cls = [mk(c) for c in range(EC)]
            else:
                w = w_sb[wtag + "8"]

                def mk8(ci):
                    def f():
                        if ci == 0:
                            cell["ps"] = psP.tile([128, 512], F32, tag="psP", name="psp_t")
                        nc.tensor.matmul(
                            cell["ps"][:],
                            lhsT=w[:, ci * 4 * D : (ci + 1) * 4 * D].rearrange(
                                "p (two f) -> p two f", two=2
                            ),
                            rhs=src_v[:, 2 * ci : 2 * ci + 2, :],
                            start=(ci == 0),
                            stop=(ci == EC // 2 - 1),
                            perf_mode=DR,
                        )
                        if ci == EC // 2 - 1:
                            nc.vector.tensor_copy(
                                outT_sb[:, col0 : col0 + 512], cell["ps"][:]
                            )
                    return f

                cls = [mk8(ci) for ci in range(EC // 2)]
            return cls

        def v_closures(st):
            # vh^T[d, krow] for stage st with wv stationary, then PE-transpose
            # each 128-row block into vh[krow, d] (bf16 + fp8 copies)
            cell = {}
            cls = []
            if _vdt(st) == BF16:

                def mkv(c):
                    def f():
                        if c == 0:
                            cell["ps"] = psT.tile([64, 512], F32, tag="psT", name="pst_t")
                        nc.tensor.matmul(
                            cell["ps"][:],
                            lhsT=w_sb["wv"][:, c * D : (c + 1) * D],
                            rhs=vv[st][:, c, :],
                            start=(c == 0),
                            stop=(c == EC - 1),
                        )
                        if c == EC - 1:
                            nc.vector.tensor_copy(
                                vhT_sb[:, st * 512 : st * 512 + 512], cell["ps"][:]
                            )
                    return f

                cls = [mkv(c) for c in range(EC)]
            else:

                def mkv8(ci):
                    def f():
                        if ci == 0:
                            cell["ps"] = psT.tile([64, 512], F32, tag="psT", name="pst_t")
                        nc.tensor.matmul(
                            cell["ps"][:],
                            lhsT=w_sb["wv8"][
                                :, ci * 2 * D : (ci + 1) * 2 * D
                            ].rearrange("p (two f) -> p two f", two=2),
                            rhs=vv[st][:, 2 * ci : 2 * ci + 2, :],
                            start=(ci == 0),
                            stop=(ci == EC // 2 - 1),
                            perf_mode=DR,
                        )
                        if ci == EC // 2 - 1:
                            nc.vector.tensor_copy(
                                vhT_sb[:, st * 512 : st * 512 + 512], cell["ps"][:]
                            )
                    return f

                cls = [mkv8(ci) for ci in range(EC // 2)]

            def mkt(jj):
                def f():
                    blk = 4 * st + jj
                    px = psX.tile([128, 64], BF16, tag="psX")
                    nc.tensor.matmul(
                        px[:],
                        lhsT=vhT_sb[:, blk * 128 : (blk + 1) * 128],
                        rhs=idn_sb[:],
                        is_transpose=True,
                    )
                    nc.vector.tensor_copy(
                        vh_sb[:, blk * DVP : blk * DVP + D], px[:]
                    )
                    nc.vector.tensor_copy(
                        vh8_sb[:, blk * DVP : blk * DVP + D], px[:]
                    )
                return f

            cls += [mkt(jj) for jj in range(4)]
            return cls

        def warmup(n):
            # dummy matmuls on a zeroed scratch tile: ramps the PE p-state
            # (full clock needs ~3us of continuous busy) while the first
            # input granules stream in
            sc = ptp.tile([128, 1024], BF16, tag="pt")
            nc.vector.memset(sc[:], 0.0)
            ps = psS.tile([128, 1024], F32, tag="psS")
            for _ in range(n):
                nc.tensor.matmul(
                    ps[:, 0:512],
                    lhsT=sc[:, 0:128],
                    rhs=sc[:, 0:512],
                    start=True,
                    stop=True,
                )

        def attend(g, mid=None, fill=0):
            npairs = extents[g] // 2
            ps_o = psO.tile([128, 512], F32, tag="psO")
            qlo = qhT_sb[0:64, g * 512 : (g + 1) * 512]
            qhi = qhT_sb[64:128, g * 512 : (g + 1) * 512]
            # diagonal pair first: its exp->mask->AV latency overlaps the
            # other pairs' score matmuls instead of gating the next attend
            order = [npairs - 1] + list(range(npairs - 1))
            for idx, pr in enumerate(order):
                first = idx == 0
                last = idx == len(order) - 1
                diag = causal and pr == npairs - 1
                # q-tile 0 projects through bf16 wq (pre-scaled 1/sqrt(D));
                # fp8 tiles use wq8 = 8*wq (keeps e4m3 out of subnormals),
                # so fold 1/(8*sqrt(D)) into the exp scale
                esc = 1.0 if _qdt(g) == BF16 else 1.0 / (8.0 * np.sqrt(D))
                use_bf = diag or not causal or not _avf8(g)
                ps_s = psS.tile([128, 1024], F32, tag="psS")
                for h in range(2):
                    l = 2 * pr + h
                    krows = khT_sb[0:64, :] if h == 0 else khT_sb[64:128, :]
                    nc.tensor.matmul(
                        ps_s[:, h * 512 : (h + 1) * 512],
                        lhsT=krows[:, l * 128 : (l + 1) * 128],
                        rhs=(qlo if h == 0 else qhi),
                        start=True,
                        stop=True,
                    )
                if use_bf:
                    # bf16 path (mask applies post-exp as 0/1 multiply)
                    pt = ptp.tile([128, 1024], BF16, tag="pt")
                    nc.scalar.activation(
                        pt[:], ps_s[:], AF.Exp, bias=bias_sb[:], scale=esc
                    )
                    if diag:
                        nc.vector.tensor_mul(pt[:], pt[:], msk_sb[:])
                    if mid is not None and idx == 0:
                        mid()
                        mid = None
                    drain(fill)
                    for h in range(2):
                        l = 2 * pr + h
                        nc.tensor.matmul(
                            ps_o[:],
                            lhsT=vh_sb[:, l * DVP : (l + 1) * DVP],
                            rhs=pt[:, h * 512 : (h + 1) * 512],
                            start=(first and h == 0),
                            stop=(last and h == 1),
                        )
                else:
                    # off-diagonal: fp8 P, one DoubleRow AV for the pair
                    pt8 = ptp8.tile([128, 1024], FP8, tag="pt8")
                    nc.scalar.activation(
                        pt8[:], ps_s[:], AF.Exp, bias=bias_sb[:], scale=esc
                    )
                    drain(fill)
                    nc.tensor.matmul(
                        ps_o[:],
                        lhsT=vh8_sb[
                            :, (2 * pr) * DVP : (2 * pr + 2) * DVP
                        ].rearrange("p (two f) -> p two f", two=2),
                        rhs=pt8[:].rearrange("p (two f) -> p two f", two=2),
                        start=first,
                        stop=last,
                        perf_mode=DR,
                    )
            ob = obp.tile([65, 512], F32, tag="ob")
            nc.vector.tensor_copy(ob[:], ps_o[0:65, :])
            nc.gpsimd.dma_start(o[:, g * 512 : (g + 1) * 512], ob[:])

        def push_stage(st):
            # stage st's kh/vh projections + its two q-tile projections
            fill_q.extend(qk_closures(st, kv[st], "wk", khT_sb, st * 512))
            fill_q.extend(v_closures(st))
            for g in (2 * st, 2 * st + 1):
                fill_q.extend(qk_closures(g, qv[g], "wq", qhT_sb, g * 512))

        if causal:
            # Get the first exp onto Activation ASAP: kh0/qh0/scores(0)
            # before the V-projection (whose v0 DMA lands later); V-proj
            # slots between exp(0) and AV(0).  Stages 2-3's projection
            # bursts (whose granules are guaranteed to have streamed in by
            # then) drain as PE fillers inside earlier attends' exp slots
            # so the Activation stream doesn't pause at stage boundaries;
            # stage 1 stays inline (its data is still streaming).
            inline(qk_closures(0, kv[0], "wk", khT_sb, 0))
            inline(qk_closures(0, qv[0], "wq", qhT_sb, 0))
            attend(0, mid=lambda: inline(v_closures(0)))
            inline(qk_closures(1, qv[1], "wq", qhT_sb, 512))
            attend(1)
            inline(qk_closures(1, kv[1], "wk", khT_sb, 512))
            inline(v_closures(1))
            inline(qk_closures(2, qv[2], "wq", qhT_sb, 1024))
            inline(qk_closures(3, qv[3], "wq", qhT_sb, 1536))
            push_stage(2)
            attend(2, fill=3)
            attend(3, fill=4)
            drain()
            push_stage(3)
            attend(4, fill=3)
            attend(5, fill=3)
            drain()
            attend(6)
            attend(7)
        else:
            for st in range(NST):
                inline(qk_closures(st, kv[st], "wk", khT_sb, st * 512))
                inline(v_closures(st))
            for g in range(NQT):
                inline(qk_closures(g, qv[g], "wq", qhT_sb, g * 512))
                attend(g)

    _split_sync_waits(nc)
    return nc


_CACHE = {}


def _get_nc(causal):
    key = bool(causal)
    if key not in _CACHE:
        extents = [2 * g + 2 for g in range(NQT)] if causal else [NKB_LOCAL] * NQT
        _CACHE[key] = build_nc(extents, causal=key)
    return _CACHE[key]


def _pack(block512, np_dt):
    # [512, E] -> [128, 4096] with out[p, c*512+s] = block[s, c*128+p]
    return np.ascontiguousarray(
        block512.reshape(512, EC, 128).transpose(2, 1, 0).reshape(128, 4096)
    ).astype(np_dt)


def _np_qdt(g):
    return BF16NP if g == 0 else FP8E3NP


def _np_kdt(t):
    return BF16NP if t == 0 else FP8NP


def _np_vdt(t):
    return BF16NP if t <= 1 else FP8NP


def kernel(q, k, v, mask, wq, wk, wv):
    q = np.asarray(q, np.float32)
    k = np.asarray(k, np.float32)
    v = np.asarray(v, np.float32)
    mask = np.asarray(mask)
    wq = np.asarray(wq, np.float32)
    wk = np.asarray(wk, np.float32)
    wv = np.asarray(wv, np.float32)

    m0 = mask[0]
    causal = bool(m0[0, 1] == 0)
    tril = np.tril(np.ones((S, S), np.int32))
    if causal:
        ok = np.array_equal(m0.astype(np.int32), tril)
    else:
        ok = bool((m0 != 0).all())
    if not ok:
        # arbitrary mask: bail out to exact numpy (correctness safety net)
        qh = q @ wq
        kh = k @ wk
        vh = v @ wv
        s = np.einsum("bqd,bkd->bqk", qh, kh) / np.sqrt(D)
        s = np.where(mask == 0, -np.inf, s)
        s = s - s.max(-1, keepdims=True)
        p = np.exp(s)
        p /= p.sum(-1, keepdims=True)
        return np.einsum("bqk,bkd->bqd", p, vh).astype(np.float32)

    nc = _get_nc(causal)

    def wchunk(w, dup, np_dt):
        # [E, D] -> [128, EC*(2)D] with w_r[p, c*D+d] = w[c*128+p, d]
        r = w.reshape(EC, 128, D).transpose(1, 0, 2)
        if dup:
            r = np.concatenate([r, r], axis=2)
        return np.ascontiguousarray(r.reshape(128, -1)).astype(np_dt)

    consts = {
        "wq": wchunk(wq / np.sqrt(D), True, BF16NP),
        "wk": wchunk(wk, True, BF16NP),
        "wv": wchunk(wv, False, BF16NP),
        # wq8 is scaled UP by 8 (not down by 1/sqrt(D)): wq/8 — and even
        # raw wq — sits partly in e4m3's subnormal range.  The 1/(8*8) is
        # folded into the exp() scale for fp8-projected q-tiles.
        "wq8": wchunk(wq * 8.0, True, FP8NP),
        "wk8": wchunk(wk, True, FP8NP),
        "wv8": wchunk(wv, False, FP8NP),
        "idn": np.eye(64, dtype=BF16NP),
    }

    in_maps = []
    for b in range(B):
        qt = {
            f"q{g}": _pack(q[b][g * 512 : (g + 1) * 512], _np_qdt(g))
            for g in range(NQT)
        }
        for p in range(2):
            kb = k[b].reshape(32, 128, E)[p::2]
            vb = v[b].reshape(32, 128, E)[p::2]
            kt = {
                f"k{t}": _pack(kb[4 * t : 4 * t + 4].reshape(512, E), _np_kdt(t))
                for t in range(NST)
            }
            vt = {
                f"v{t}": _pack(vb[4 * t : 4 * t + 4].reshape(512, E), _np_vdt(t))
                for t in range(NST)
            }
            if causal:
                kk = np.arange(128)[:, None]
                qq = np.arange(512)[None, :]
                parts = []
                for j in (p, p + 2):
                    allowed = qq >= (j * 128 + kk)
                    parts.append(np.where(allowed, 1.0, 0.0).astype(BF16NP))
                mskd = np.concatenate(parts, axis=1)  # [128, 1024] of 1/0
            else:
                mskd = np.ones((128, 1024), BF16NP)
            im = dict(consts)
            im["msk"] = mskd
            im.update(qt)
            im.update(kt)
            im.update(vt)
            in_maps.append(im)

    globals()["_last_in_maps"] = in_maps
    res = run_bass_kernel_spmd(nc, in_maps, core_ids=list(range(8)))

    out = np.empty((B, S, D), np.float32)
    for b in range(B):
        oe = res.results[2 * b]["o"]    # [65, 4096]
        oo = res.results[2 * b + 1]["o"]
        num = oe[:D] + oo[:D]           # [64, 4096]
        den = oe[D] + oo[D]             # [4096]
        out[b] = (num / den).T
    return out


# revision 24
# speedup vs baseline: 1.2271x; 1.0450x over previous
"""Causal single-head attention on 8 TRN2 NeuronCores — hybrid bf16/fp8.

Math (per batch b):
    qh = q @ (wq/8); kh = k @ wk; vh = v @ wv
    S^T[k,q] = kh qh^T            (scores transposed: k on partitions)
    P^T = exp(S^T - 2) * diagmask (constant bias keeps P in fp8e4 range;
                                   it cancels exactly in num/den)
    oT[d,q] = sum_k vh_ext[k,d]^T P^T[k,q]   with vh_ext = [vh | ones]
    row 64 of oT is the softmax denominator; host divides.

Sharding: 8 cores = 4 batches x 2 k-parities (flash-decoding style).
Core (b, p) handles batch b and the interleaved k-blocks {p, p+2, ...}
(128-row blocks): q-tile g sees 2g+2 local k-blocks; the last pair
crosses the diagonal and is masked.  Each core returns oT [65, 4096];
the host sums the parity partials and divides by the denominator row.

Perf structure (v5):
  * Inputs arrive as 24 large consumer-granule DMAs (contiguous 4-8KB
    per-partition lines) in exact consumption order on one HWDGE queue
    (the fp32 baseline issued 160 small DMAs at ~600ns of sequencer
    time each and was DMA-issue-bound).  Everything is SBUF-resident.
  * Mixed precision, validated against the max-abs-rel metric: the
    first q-tile / k-stage / v-stage (global rows < 512..1023, where
    softmax averaging is weakest) runs fully in bf16; later granules
    ship as fp8e4 and their projections contract chunk-PAIRS with
    DoubleRow matmuls (2 rows/cycle).  Scores stay bf16.  Attention*V
    uses one fp8 DoubleRow matmul per off-diagonal k-block pair
    (P and vh quantized to fp8 — softmax averaging damps this to
    ~1e-2 worst-case vs the 2e-2 gate); diagonal pairs stay bf16.
  * V-projection computes vh^T with the weight stationary (32 wide
    matmuls instead of 128 N=64 ones) and PE-transposes the result.
  * Within an attend the DIAGONAL pair is emitted first so its longer
    exp->mask->AV chain overlaps the other pairs' score matmuls
    instead of gating the next attend's PSUM reuse.
  * exp() on Activation; mask is a post-exp 0/1 multiply (Vector);
    output stores ride the GpSimd queue.  GpSimd cannot touch PSUM on
    TRN2, so all PSUM evictions stay on Vector.
"""

import sys

sys.path.insert(0, "/opt/trn_rl_repo")

import numpy as np
import ml_dtypes
from contextlib import ExitStack

import concourse.bass as bass
import concourse.mybir as mybir
import concourse.tile as tile
from concourse.bass_utils import run_bass_kernel_spmd

F32 = mybir.dt.float32
BF16 = mybir.dt.bfloat16
FP8 = mybir.dt.float8e4
FP8E3 = mybir.dt.float8e3
AF = mybir.ActivationFunctionType
DR = mybir.MatmulPerfMode.DoubleRow
BF16NP = ml_dtypes.bfloat16
FP8NP = ml_dtypes.float8_e4m3
FP8E3NP = ml_dtypes.float8_e3m4

B, S, E, D = 4, 4096, 1024, 64
NQT = S // 512          # 8 q-tiles of 512 rows
NST = 4                 # k/v staged in 4 chunks of 512 local rows
NKB_LOCAL = 16          # local (per-parity) 128-row k-blocks
EC = E // 128           # 8 e-chunks
DV = D + 1              # vh width incl. ones column
DVP = 128               # padded vh block pitch: [vh(64) | ones | 63 zeros]
                        # (dual-fp8 ldweights requires subtile M in {32,64,128})
EXP_BIAS = -2.0         # P' = exp(s-2): keeps P < 240 (fp8e4 max); cancels


def _patch_tile_drain():
    """Walrus in this container rejects >1 sync-wait on a Drain instruction.
    Spread the tail drain's waits across multiple drains (idempotent; the
    following all_engine_barrier orders everything)."""
    if getattr(tile.TileContext, "_drain_patched", False):
        return
    from concourse.tile import ScopedClock

    def _split_drain_and_barrier(self, tick_clock, wait_clock):
        drain_inst = self.nc.sync.drain()
        wait_clock.add_sem_waits(
            drain_inst.ins, ScopedClock({None: tick_clock.global_clock})
        )
        mi = drain_inst.ins
        si = mi.sync_info
        if si is not None and si.on_wait and len(si.on_wait) > 1:
            waits = list(si.on_wait)
            si.on_wait = waits[:1]
            for w in waits[1:]:
                d2 = self.nc.sync.drain().ins
                si2 = d2.sync_info
                if si2 is None:
                    d2.sync_info = mybir.SyncInfo(on_wait=[w], on_update=[])
                else:
                    si2.on_wait = list(si2.on_wait) + [w]
        self.nc.all_engine_barrier()
        assert self.sems is not None
        popped = self.nc._tile_sem_poison_stack.pop()
        assert popped is self._sem_poison
        self.nc.clear_and_free_semaphores(list(self.sems.allocated().values()))
        self.nc.all_engine_barrier()

    tile.TileContext._drain_and_barrier = _split_drain_and_barrier
    tile.TileContext._drain_patched = True


WAIT_LIMIT = 1


def _split_sync_waits(nc, limit=WAIT_LIMIT):
    """This container's walrus rejects instructions carrying more than ~limit
    sem waits. Hoist excess waits onto same-engine NoOps inserted just before
    the instruction (engine streams are in-order, so the waits still gate)."""
    n_nops = 0
    for f in nc.m.functions:
        for bb in f.blocks:
            il = bb.instructions
            i = 0
            while i < len(il):
                ins = il[i]
                si = ins.sync_info
                if si is not None and si.on_wait and len(si.on_wait) > limit:
                    waits = list(si.on_wait)
                    keep = waits[-limit:]
                    excess = waits[:-limit]
                    pos = i
                    for j in range(0, len(excess), limit):
                        nop = mybir.InstNoOp(
                            name=f"{ins.name}_wsplit{j}", ins=[], outs=[]
                        )
                        nop.engine = ins.engine
                        nop.sync_info = mybir.SyncInfo(
                            on_wait=excess[j : j + limit], on_update=[]
                        )
                        il.insert(pos, nop)
                        pos += 1
                        i += 1
                        n_nops += 1
                    si.on_wait = keep
                i += 1
    return n_nops


def _qdt(g):
    # q granules: tile 0 bf16; the rest fp8-e3m4 (4 mantissa bits — q input
    # error is a rank-64 perturbation of the row's whole score vector and
    # does NOT average away with depth, so q needs the extra mantissa)
    return BF16 if g == 0 else FP8E3


def _kdt(t):
    # k error DOES average away under softmax: e4m3 + DoubleRow is fine
    return BF16 if t == 0 else FP8


def _vdt(t):
    # v feeds the output directly; keep 2 stages in bf16
    return BF16 if t <= 1 else FP8


def _avf8(g):
    # fp8 DoubleRow AV only from q-tile 2 on (rows >= 1024)
    return g >= 2


def build_nc(extents, causal=True):
    """One SPMD program; per-core data differences live in the inputs.

    extents[g] = number of local 128-row k-blocks q-tile g attends to
    (always even: causal -> 2g+2, full -> 16).

    DRAM granules ([p, c*512+s] packing, c = e-chunk, s = row-in-granule):
      q{g} [128, 4096]  q-tile g    (g==0 bf16, else fp8)
      k{t} [128, 4096]  k-stage t   (t==0 bf16, else fp8)
      v{t} [128, 4096]  v-stage t   (t==0 bf16, else fp8)
    """
    _patch_tile_drain()
    nc = bass.Bass("TRN2", target_bir_lowering=False)

    qd = [nc.dram_tensor(f"q{g}", [128, 4096], _qdt(g), kind="ExternalInput")
          for g in range(NQT)]
    kd = [nc.dram_tensor(f"k{t}", [128, 4096], _kdt(t), kind="ExternalInput")
          for t in range(NST)]
    vd = [nc.dram_tensor(f"v{t}", [128, 4096], _vdt(t), kind="ExternalInput")
          for t in range(NST)]
    # bf16 weights, chunked: w_r[p, c*D+d] = w[c*128+p, d] (q/k duplicated)
    wq = nc.dram_tensor("wq", [128, EC * 2 * D], BF16, kind="ExternalInput")
    wk = nc.dram_tensor("wk", [128, EC * 2 * D], BF16, kind="ExternalInput")
    wv = nc.dram_tensor("wv", [128, EC * D], BF16, kind="ExternalInput")
    # fp8 copies (same layout; DoubleRow consumes chunk pairs)
    wq8 = nc.dram_tensor("wq8", [128, EC * 2 * D], FP8, kind="ExternalInput")
    wk8 = nc.dram_tensor("wk8", [128, EC * 2 * D], FP8, kind="ExternalInput")
    wv8 = nc.dram_tensor("wv8", [128, EC * D], FP8, kind="ExternalInput")
    idn = nc.dram_tensor("idn", [64, 64], BF16, kind="ExternalInput")
    msk = nc.dram_tensor("msk", [128, 1024], BF16, kind="ExternalInput")
    o = nc.dram_tensor("o", [DV, S], F32, kind="ExternalOutput")

    with tile.TileContext(nc) as tc, ExitStack() as ctx:
        const = ctx.enter_context(tc.tile_pool(name="const", bufs=1))
        xin = ctx.enter_context(tc.tile_pool(name="xin", bufs=1))
        big = ctx.enter_context(tc.tile_pool(name="big", bufs=1))
        ptp = ctx.enter_context(tc.tile_pool(name="ptp", bufs=2))
        ptp8 = ctx.enter_context(tc.tile_pool(name="ptp8", bufs=3))
        obp = ctx.enter_context(tc.tile_pool(name="obp", bufs=2))

        psP = ctx.enter_context(tc.tile_pool(name="psP", bufs=1, space="PSUM"))
        psS = ctx.enter_context(tc.tile_pool(name="psS", bufs=2, space="PSUM"))
        psO = ctx.enter_context(tc.tile_pool(name="psO", bufs=1, space="PSUM"))
        psT = ctx.enter_context(tc.tile_pool(name="psT", bufs=1, space="PSUM"))
        psX = ctx.enter_context(tc.tile_pool(name="psX", bufs=1, space="PSUM"))

        # --- SBUF homes -------------------------------------------------
        w_sb = {}
        for name, dram, wd, dt_ in (
            ("wk", wk, 2, BF16), ("wq", wq, 2, BF16), ("wv", wv, 1, BF16),
            ("wk8", wk8, 2, FP8), ("wq8", wq8, 2, FP8), ("wv8", wv8, 1, FP8),
        ):
            w_sb[name] = const.tile(
                [128, EC * wd * D], dt_, tag=f"w_{name}", name=f"w_{name}_sb"
            )
        idn_sb = const.tile([64, 64], BF16, tag="idn")
        msk_sb = const.tile([128, 1024], BF16, tag="msk")
        bias_sb = const.tile([128, 1], F32, tag="bias")
        nc.vector.memset(bias_sb[:], EXP_BIAS)
        q_sb = [
            xin.tile([128, 4096], _qdt(g), tag=f"q{g}", name=f"q{g}_sb")
            for g in range(NQT)
        ]
        k_sb = [
            xin.tile([128, 4096], _kdt(t), tag=f"k{t}", name=f"k{t}_sb")
            for t in range(NST)
        ]
        v_sb = [
            xin.tile([128, 4096], _vdt(t), tag=f"v{t}", name=f"v{t}_sb")
            for t in range(NST)
        ]

        # --- input DMAs, one per granule, in consumption order ----------
        dma = nc.sync.dma_start
        dma(w_sb["wq"][:], wq[:])
        dma(w_sb["wk"][:], wk[:])
        if causal:
            dma(q_sb[0][:], qd[0][:])
            dma(k_sb[0][:], kd[0][:])
            dma(msk_sb[:], msk[:])
            dma(w_sb["wv"][:], wv[:])
            dma(idn_sb[:], idn[:])
            dma(v_sb[0][:], vd[0][:])
            dma(w_sb["wq8"][:], wq8[:])
            dma(q_sb[1][:], qd[1][:])
            dma(w_sb["wk8"][:], wk8[:])
            dma(w_sb["wv8"][:], wv8[:])
            for st in range(1, NST):
                dma(k_sb[st][:], kd[st][:])
                dma(v_sb[st][:], vd[st][:])
                dma(q_sb[2 * st][:], qd[2 * st][:])
                dma(q_sb[2 * st + 1][:], qd[2 * st + 1][:])
        else:
            dma(w_sb["wv"][:], wv[:])
            dma(idn_sb[:], idn[:])
            dma(msk_sb[:], msk[:])
            dma(w_sb["wk8"][:], wk8[:])
            dma(w_sb["wq8"][:], wq8[:])
            dma(w_sb["wv8"][:], wv8[:])
            for t in range(NST):
                dma(k_sb[t][:], kd[t][:])
                dma(v_sb[t][:], vd[t][:])
            for g in range(NQT):
                dma(q_sb[g][:], qd[g][:])

        # [p, c, s] views of the packed granules
        qv = [t[:].rearrange("p (c s) -> p c s", s=512) for t in q_sb]
        kv = [t[:].rearrange("p (c s) -> p c s", s=512) for t in k_sb]
        vv = [t[:].rearrange("p (c s) -> p c s", s=512) for t in v_sb]

        qhT_sb = big.tile([128, S], BF16, tag="qhT")
        khT_sb = big.tile([128, S // 2], BF16, tag="khT")
        vhT_sb = big.tile([64, NKB_LOCAL * 128], BF16, tag="vhT")
        vh_sb = big.tile([128, NKB_LOCAL * DVP], BF16, tag="vh")
        vh8_sb = big.tile([128, NKB_LOCAL * DVP], FP8, tag="vh8")
        # col D = ones (softmax denominator via the AV matmul); cols D+1..
        # are never read but are zeroed so stale fp8 bytes can't be NaN/inf
        for t in (vh_sb, vh8_sb):
            v3 = t[:].rearrange("p (b c) -> p b c", c=DVP)
            nc.gpsimd.memset(v3[:, :, D], 1.0)
            nc.gpsimd.memset(v3[:, :, D + 1 :], 0.0)

        # ---- projections as closure lists so they can be interleaved into
        # ---- attend pair slots (keeps the PE dense while ACT runs exp) ----
        fill_q = []

        def drain(n=None):
            k2 = 0
            while fill_q and (n is None or k2 < n):
                fill_q.pop(0)()
                k2 += 1

        def inline(cls):
            for f in cls:
                f()

        def qk_closures(i, src_v, wtag, outT_sb, col0):
            # outT[128, col0:+512] = (x @ [w|w])^T, contracting E.
            # q fp8 granules are e3m4: plain matmuls against e4m3 weights
            # (e3m4 has no DoubleRow); k fp8 granules are e4m3: chunk-pair
            # DoubleRow.
            dt_ = _qdt(i) if wtag == "wq" else _kdt(i)
            cell = {}
            cls = []
            if dt_ != FP8:
                w = w_sb[wtag] if dt_ == BF16 else w_sb[wtag + "8"]

                def mk(c):
                    def f():
                        if c == 0:
                            cell["ps"] = psP.tile([128, 512], F32, tag="psP", name="psp_t")
                        nc.tensor.matmul(
                            cell["ps"][:],
                            lhsT=w[:, c * 2 * D : (c + 1) * 2 * D],
                            rhs=src_v[:, c, :],
                            start=(c == 0),
                            stop=(c == EC - 1),
                        )
                        if c == EC - 1:
                            nc.vector.tensor_copy(
                                outT_sb[:, col0 : col0 + 512], cell["ps"][:]
                            )
                    return f

                cls = [mk(c) for c in range(EC)]
            else:
                w = w_sb[wtag + "8"]

                def mk8(ci):
                    def f():
                        if ci == 0:
                            cell["ps"] = psP.tile([128, 512], F32, tag="psP", name="psp_t")
                        nc.tensor.matmul(
                            cell["ps"][:],
                            lhsT=w[:, ci * 4 * D : (ci + 1) * 4 * D].rearrange(
                                "p (two f) -> p two f", two=2
                            ),
                            rhs=src_v[:, 2 * ci : 2 * ci + 2, :],
                            start=(ci == 0),
                            stop=(ci == EC // 2 - 1),
                            perf_mode=DR,
                        )
                        if ci == EC // 2 - 1:
                            nc.vector.tensor_copy(
                                outT_sb[:, col0 : col0 + 512], cell["ps"][:]
                            )
                    return f

                cls = [mk8(ci) for ci in range(EC // 2)]
            return cls

        def v_closures(st):
            # vh^T[d, krow] for stage st with wv stationary, then PE-transpose
            # each 128-row block into vh[krow, d] (bf16 + fp8 copies)
            cell = {}
            cls = []
            if _vdt(st) == BF16:

                def mkv(c):
                    def f():
                        if c == 0:
                            cell["ps"] = psT.tile([64, 512], F32, tag="psT", name="pst_t")
                        nc.tensor.matmul(
                            cell["ps"][:],
                            lhsT=w_sb["wv"][:, c * D : (c + 1) * D],
                            rhs=vv[st][:, c, :],
                            start=(c == 0),
                            stop=(c == EC - 1),
                        )
                        if c == EC - 1:
                            nc.vector.tensor_copy(
                                vhT_sb[:, st * 512 : st * 512 + 512], cell["ps"][:]
                            )
                    return f

                cls = [mkv(c) for c in range(EC)]
            else:

                def mkv8(ci):
                    def f():
                        if ci == 0:
                            cell["ps"] = psT.tile([64, 512], F32, tag="psT", name="pst_t")
                        nc.tensor.matmul(
                            cell["ps"][:],
                            lhsT=w_sb["wv8"][
                                :, ci * 2 * D : (ci + 1) * 2 * D
                            ].rearrange("p (two f) -> p two f", two=2),
                            rhs=vv[st][:, 2 * ci : 2 * ci + 2, :],
                            start=(ci == 0),
                            stop=(ci == EC // 2 - 1),
                            perf_mode=DR,
                        )
                        if ci == EC // 2 - 1:
                            nc.vector.tensor_copy(
                                vhT_sb[:, st * 512 : st * 512 + 512], cell["ps"][:]
                            )
                    return f

                cls = [mkv8(ci) for ci in range(EC // 2)]

            def mkt(jj):
                def f():
                    blk = 4 * st + jj
                    px = psX.tile([128, 64], BF16, tag="psX")
                    nc.tensor.matmul(
                        px[:],
                        lhsT=vhT_sb[:, blk * 128 : (blk + 1) * 128],
                        rhs=idn_sb[:],
                        is_transpose=True,
                    )
                    nc.vector.tensor_copy(
                        vh_sb[:, blk * DVP : blk * DVP + D], px[:]
                    )
                    nc.vector.tensor_copy(
                        vh8_sb[:, blk * DVP : blk * DVP + D], px[:]
                    )
                return f

            cls += [mkt(jj) for jj in range(4)]
            return cls

        def warmup(n):
            # dummy matmuls on a zeroed scratch tile: ramps the PE p-state
            # (full clock needs ~3us of continuous busy) while the first
            # input granules stream in
            sc = ptp.tile([128, 1024], BF16, tag="pt")
            nc.vector.memset(sc[:], 0.0)
            ps = psS.tile([128, 1024], F32, tag="psS")
            for _ in range(n):
                nc.tensor.matmul(
                    ps[:, 0:512],
                    lhsT=sc[:, 0:128],
                    rhs=sc[:, 0:512],
                    start=True,
                    stop=True,
                )

        def attend(g, mid=None, fill=0):
            npairs = extents[g] // 2
            ps_o = psO.tile([128, 512], F32, tag="psO")
            qlo = qhT_sb[0:64, g * 512 : (g + 1) * 512]
            qhi = qhT_sb[64:128, g * 512 : (g + 1) * 512]
            # diagonal pair first: its exp->mask->AV latency overlaps the
            # other pairs' score matmuls instead of gating the next attend
            order = [npairs - 1] + list(range(npairs - 1))
            for idx, pr in enumerate(order):
                first = idx == 0
                last = idx == len(order) - 1
                diag = causal and pr == npairs - 1
                # q-tile 0 projects through bf16 wq (pre-scaled 1/sqrt(D));
                # fp8 tiles use wq8 = 8*wq (keeps e4m3 out of subnormals),
                # so fold 1/(8*sqrt(D)) into the exp scale
                esc = 1.0 if _qdt(g) == BF16 else 1.0 / (8.0 * np.sqrt(D))
                use_bf = diag or not causal or not _avf8(g)
                ps_s = psS.tile([128, 1024], F32, tag="psS")
                for h in range(2):
                    l = 2 * pr + h
                    krows = khT_sb[0:64, :] if h == 0 else khT_sb[64:128, :]
                    nc.tensor.matmul(
                        ps_s[:, h * 512 : (h + 1) * 512],
                        lhsT=krows[:, l * 128 : (l + 1) * 128],
                        rhs=(qlo if h == 0 else qhi),
                        start=True,
                        stop=True,
                    )
                if use_bf:
                    # bf16 path (mask applies post-exp as 0/1 multiply)
                    pt = ptp.tile([128, 1024], BF16, tag="pt")
                    nc.scalar.activation(
                        pt[:], ps_s[:], AF.Exp, bias=bias_sb[:], scale=esc
                    )
                    if diag:
                        nc.vector.tensor_mul(pt[:], pt[:], msk_sb[:])
                    if mid is not None and idx == 0:
                        mid()
                        mid = None
                    drain(fill)
                    for h in range(2):
                        l = 2 * pr + h
                        nc.tensor.matmul(
                            ps_o[:],
                            lhsT=vh_sb[:, l * DVP : (l + 1) * DVP],
                            rhs=pt[:, h * 512 : (h + 1) * 512],
                            start=(first and h == 0),
                            stop=(last and h == 1),
                        )
                else:
                    # off-diagonal: fp8 P, one DoubleRow AV for the pair
                    pt8 = ptp8.tile([128, 1024], FP8, tag="pt8")
                    nc.scalar.activation(
                        pt8[:], ps_s[:], AF.Exp, bias=bias_sb[:], scale=esc
                    )
                    drain(fill)
                    nc.tensor.matmul(
                        ps_o[:],
                        lhsT=vh8_sb[
                            :, (2 * pr) * DVP : (2 * pr + 2) * DVP
                        ].rearrange("p (two f) -> p two f", two=2),
                        rhs=pt8[:].rearrange("p (two f) -> p two f", two=2),
                        start=first,
                        stop=last,
                        perf_mode=DR,
                    )
            ob = obp.tile([65, 512], F32, tag="ob")
            nc.vector.tensor_copy(ob[:], ps_o[0:65, :])
            nc.gpsimd.dma_start(o[:, g * 512 : (g + 1) * 512], ob[:])

        def push_stage(st):
            # stage st's kh/vh projections + its two q-tile projections
            fill_q.extend(qk_closures(st, kv[st], "wk", khT_sb, st * 512))
            fill_q.extend(v_closures(st))
            for g in (2 * st, 2 * st + 1):
                fill_q.extend(qk_closures(g, qv[g], "wq", qhT_sb, g * 512))

        def kh0_half(h):
            # stage-0 kh in two column halves: attend(0) only needs k-blocks
            # 0-1 (cols 0:256), so the first exp unlocks ~2us earlier
            ps = psP.tile([128, 512], F32, tag="psP", name="psp_h")
            for c in range(EC):
                nc.tensor.matmul(
                    ps[:, h * 256 : (h + 1) * 256],
                    lhsT=w_sb["wk"][:, c * 2 * D : (c + 1) * 2 * D],
                    rhs=kv[0][:, c, h * 256 : (h + 1) * 256],
                    start=(c == 0),
                    stop=(c == EC - 1),
                )
            nc.vector.tensor_copy(
                khT_sb[:, h * 256 : (h + 1) * 256], ps[:, h * 256 : (h + 1) * 256]
            )

        if causal:
            # Get the first exp onto Activation ASAP: qh0 (its DMA lands
            # first), then only the diagonal half of kh0, scores(0); the
            # V-projection (whose v0 DMA lands later) slots between exp(0)
            # and AV(0), and the second kh0 half rides along with it.
            inline(qk_closures(0, qv[0], "wq", qhT_sb, 0))
            kh0_half(0)
            attend(0, mid=lambda: (kh0_half(1), inline(v_closures(0))))
            inline(qk_closures(1, qv[1], "wq", qhT_sb, 512))
            attend(1)
            for st in range(1, NST):
                inline(qk_closures(st, kv[st], "wk", khT_sb, st * 512))
                inline(v_closures(st))
                for g in (2 * st, 2 * st + 1):
                    inline(qk_closures(g, qv[g], "wq", qhT_sb, g * 512))
                    attend(g)
        else:
            for st in range(NST):
                inline(qk_closures(st, kv[st], "wk", khT_sb, st * 512))
                inline(v_closures(st))
            for g in range(NQT):
                inline(qk_closures(g, qv[g], "wq", qhT_sb, g * 512))
                attend(g)

    _split_sync_waits(nc)
    return nc


_CACHE = {}


def _get_nc(causal):
    key = bool(causal)
    if key not in _CACHE:
        extents = [2 * g + 2 for g in range(NQT)] if causal else [NKB_LOCAL] * NQT
        _CACHE[key] = build_nc(extents, causal=key)
    return _CACHE[key]


def _pack(block512, np_dt):
    # [512, E] -> [128, 4096] with out[p, c*512+s] = block[s, c*128+p]
    return np.ascontiguousarray(
        block512.reshape(512, EC, 128).transpose(2, 1, 0).reshape(128, 4096)
    ).astype(np_dt)


def _np_qdt(g):
    return BF16NP if g == 0 else FP8E3NP


def _np_kdt(t):
    return BF16NP if t == 0 else FP8NP


def _np_vdt(t):
    return BF16NP if t <= 1 else FP8NP


def kernel(q, k, v, mask, wq, wk, wv):
    q = np.asarray(q, np.float32)
    k = np.asarray(k, np.float32)
    v = np.asarray(v, np.float32)
    mask = np.asarray(mask)
    wq = np.asarray(wq, np.float32)
    wk = np.asarray(wk, np.float32)
    wv = np.asarray(wv, np.float32)

    m0 = mask[0]
    causal = bool(m0[0, 1] == 0)
    tril = np.tril(np.ones((S, S), np.int32))
    if causal:
        ok = np.array_equal(m0.astype(np.int32), tril)
    else:
        ok = bool((m0 != 0).all())
    if not ok:
        # arbitrary mask: bail out to exact numpy (correctness safety net)
        qh = q @ wq
        kh = k @ wk
        vh = v @ wv
        s = np.einsum("bqd,bkd->bqk", qh, kh) / np.sqrt(D)
        s = np.where(mask == 0, -np.inf, s)
        s = s - s.max(-1, keepdims=True)
        p = np.exp(s)
        p /= p.sum(-1, keepdims=True)
        return np.einsum("bqk,bkd->bqd", p, vh).astype(np.float32)

    nc = _get_nc(causal)

    def wchunk(w, dup, np_dt):
        # [E, D] -> [128, EC*(2)D] with w_r[p, c*D+d] = w[c*128+p, d]
        r = w.reshape(EC, 128, D).transpose(1, 0, 2)
        if dup:
            r = np.concatenate([r, r], axis=2)
        return np.ascontiguousarray(r.reshape(128, -1)).astype(np_dt)

    consts = {
        "wq": wchunk(wq / np.sqrt(D), True, BF16NP),
        "wk": wchunk(wk, True, BF16NP),
        "wv": wchunk(wv, False, BF16NP),
        # wq8 is scaled UP by 8 (not down by 1/sqrt(D)): wq/8 — and even
        # raw wq — sits partly in e4m3's subnormal range.  The 1/(8*8) is
        # folded into the exp() scale for fp8-projected q-tiles.
        "wq8": wchunk(wq * 8.0, True, FP8NP),
        "wk8": wchunk(wk, True, FP8NP),
        "wv8": wchunk(wv, False, FP8NP),
        "idn": np.eye(64, dtype=BF16NP),
    }

    in_maps = []
    for b in range(B):
        qt = {
            f"q{g}": _pack(q[b][g * 512 : (g + 1) * 512], _np_qdt(g))
            for g in range(NQT)
        }
        for p in range(2):
            kb = k[b].reshape(32, 128, E)[p::2]
            vb = v[b].reshape(32, 128, E)[p::2]
            kt = {
                f"k{t}": _pack(kb[4 * t : 4 * t + 4].reshape(512, E), _np_kdt(t))
                for t in range(NST)
            }
            vt = {
                f"v{t}": _pack(vb[4 * t : 4 * t + 4].reshape(512, E), _np_vdt(t))
                for t in range(NST)
            }
            if causal:
                kk = np.arange(128)[:, None]
                qq = np.arange(512)[None, :]
                parts = []
                for j in (p, p + 2):
                    allowed = qq >= (j * 128 + kk)
                    parts.append(np.where(allowed, 1.0, 0.0).astype(BF16NP))
                mskd = np.concatenate(parts, axis=1)  # [128, 1024] of 1/0
            else:
                mskd = np.ones((128, 1024), BF16NP)
            im = dict(consts)
            im["msk"] = mskd
            im.update(qt)
            im.update(kt)
            im.update(vt)
            in_maps.append(im)

    globals()["_last_in_maps"] = in_maps
    res = run_bass_kernel_spmd(nc, in_maps, core_ids=list(range(8)))

    out = np.empty((B, S, D), np.float32)
    for b in range(B):
        oe = res.results[2 * b]["o"]    # [65, 4096]
        oo = res.results[2 * b + 1]["o"]
        num = oe[:D] + oo[:D]           # [64, 4096]
        den = oe[D] + oo[D]             # [4096]
        out[b] = (num / den).T
    return out
